# revision 2
# baseline (speedup 1.0000x reference)
"""VQ codebook nearest-neighbor kernel for Trainium2 (8 NeuronCores, data-parallel).

Problem: z [2048,64,256] f32, E [1024,256] f32 ->
         out[b,u,:] = E[argmin_k ||z[b,u]-E[k]||^2]

Strategy (v2):
  - Shard z along batch across 8 cores (16384 tokens each); replicate E.
  - argmin_k ||z-e_k||^2 == argmax_k (z.e_k - ||e_k||^2/2).
  - Scores via TWO matmul passes instead of three:
      pass1: fp32r(z, E)       -- PE rounds both operands to 11 mantissa bits
      pass2: bf16(z) x bf16(E - r11(E))  -- corrects E's lost low bits
    (r11 = round-to-nearest at 11 explicit mantissa bits, matching the PE's
    fp32r behavior, measured on HW.  Residual z_lo x E term flips ~14 argmaxes
    of 131072; rel err ~0.016 < 2e-2 gate.  THREE_PASS adds bf16(z_lo, E_hi)
    to drop that to ~0.009.)
  - -|e_k|^2/2 bias computed exactly on host, folded in as a K=1 fp32r matmul.
  - Argmax without a second DVE reduction pass:
      DVE: prefix-max scan (tensor_tensor_scan op0=max) -> rm; rm[-1] is max
      ACT: activation(Sign, scale=-1, bias=rm[-1], accum_out) ->
           sum sign(max - rm_t) = #positions before first max = argmax index
      ACT: f32 -> u32 copy; gpsimd indirect DMA gathers E rows; store via ACT
           HWDGE queue.
"""
import numpy as np
import ml_dtypes

B, U, K, D = 2048, 64, 1024, 256
N_CORES = 8
TOK = B * U                    # 131072 tokens total
TOK_PC = TOK // N_CORES        # 16384 tokens per core
SUPER = 512                    # tokens per DMA super-tile
TILE = 128                     # tokens per compute tile
N_SUPER = TOK_PC // SUPER      # 32
TILES_PER_SUPER = SUPER // TILE  # 4

THREE_PASS = False             # safety toggle: add bf16(z_lo, E_hi) pass

_compiled = None


def _r11(x):
    """Round-to-nearest float32 keeping 11 explicit mantissa bits (PE fp32r)."""
    xi = np.ascontiguousarray(x, dtype=np.float32).view(np.uint32)
    half = np.uint32(1 << 11)
    mask = np.uint32((~((1 << 12) - 1)) & 0xFFFFFFFF)
    return ((xi + half) & mask).view(np.float32)


def _build(reps: int = 1):
    from concourse import bacc
    import concourse.mybir as mybir
    import concourse.tile as tile
    import concourse.bass as bass
    import contextlib

    f32 = mybir.dt.float32
    f32r = mybir.dt.float32r
    bf16 = mybir.dt.bfloat16
    u32 = mybir.dt.uint32
    AF = mybir.ActivationFunctionType

    nc = bacc.Bacc("TRN2", target_bir_lowering=False, debug=False,
                   num_devices=N_CORES)

    zf = nc.declare_dram_parameter("zf", [D, TOK_PC], f32r, isOutput=False)
    zh = nc.declare_dram_parameter("zh", [D, TOK_PC], bf16, isOutput=False)
    ef = nc.declare_dram_parameter("ef", [D, K], f32r, isOutput=False)
    eb = nc.declare_dram_parameter("eb", [D, K], bf16, isOutput=False)
    if THREE_PASS:
        zl = nc.declare_dram_parameter("zl", [D, TOK_PC], bf16, isOutput=False)
        eh = nc.declare_dram_parameter("eh", [D, K], bf16, isOutput=False)
    br = nc.declare_dram_parameter("br", [1, K], f32, isOutput=False)
    etab = nc.declare_dram_parameter("etab", [K, D], f32, isOutput=False)
    out = nc.declare_dram_parameter("out", [TOK_PC, D], f32, isOutput=True)

    with tile.TileContext(nc) as tc:
        with contextlib.ExitStack() as ctx:
            const = ctx.enter_context(tc.tile_pool(name="const", bufs=1))
            zpool = ctx.enter_context(tc.tile_pool(name="zp", bufs=3))
            rmpool = ctx.enter_context(tc.tile_pool(name="rm", bufs=2))
            scrpool = ctx.enter_context(tc.tile_pool(name="scr", bufs=2))
            gpool = ctx.enter_context(tc.tile_pool(name="gp", bufs=4))
            ipool = ctx.enter_context(tc.tile_pool(name="ip", bufs=4))
            psum = ctx.enter_context(tc.tile_pool(name="ps", bufs=3, space="PSUM"))

            # ---------------- one-time setup ----------------
            ef_sb = const.tile([128, 2, K], f32r, tag="efsb")
            eb_sb = const.tile([128, 2, K], bf16, tag="ebsb")
            for c in range(2):
                nc.sync.dma_start(ef_sb[:, c, :], ef[c*128:(c+1)*128, :])
                nc.sync.dma_start(eb_sb[:, c, :], eb[c*128:(c+1)*128, :])
            if THREE_PASS:
                eh_sb = const.tile([128, 2, K], bf16, tag="ehsb")
                for c in range(2):
                    nc.sync.dma_start(eh_sb[:, c, :], eh[c*128:(c+1)*128, :])
            br_sb = const.tile([1, K], f32, tag="brsb")
            nc.sync.dma_start(br_sb[:], br[:, :])
            bias_row = const.tile([1, K], f32r, tag="biasrow")
            nc.vector.tensor_copy(bias_row[:], br_sb[:])
            ones_f = const.tile([1, 128], f32, tag="onesf")
            nc.vector.memset(ones_f[:], 1.0)
            ones_row = const.tile([1, 128], f32r, tag="onesrow")
            nc.vector.tensor_copy(ones_row[:], ones_f[:])
            dummy1 = const.tile([TILE, 1], f32, tag="dummy1")
            nc.vector.memset(dummy1[:], 0.0)

            def main_loop():
                for s in range(N_SUPER):
                    z_sb = zpool.tile([128, 2, SUPER], f32r, tag="zsb")
                    zh_sb = zpool.tile([128, 2, SUPER], bf16, tag="zhsb")
                    for c in range(2):
                        nc.sync.dma_start(z_sb[:, c, :],
                                          zf[c*128:(c+1)*128, s*SUPER:(s+1)*SUPER])
                        nc.sync.dma_start(zh_sb[:, c, :],
                                          zh[c*128:(c+1)*128, s*SUPER:(s+1)*SUPER])
                    if THREE_PASS:
                        zl_sb = zpool.tile([128, 2, SUPER], bf16, tag="zlsb")
                        for c in range(2):
                            nc.sync.dma_start(zl_sb[:, c, :],
                                              zl[c*128:(c+1)*128,
                                                 s*SUPER:(s+1)*SUPER])
                    for j in range(TILES_PER_SUPER):
                        tok0 = s * SUPER + j * TILE
                        sl = slice(j*TILE, (j+1)*TILE)
                        acc = psum.tile([TILE, K], f32, tag="acc")
                        for n in range(2):
                            nsl = slice(n*512, (n+1)*512)
                            nc.tensor.matmul(acc[:, nsl], lhsT=z_sb[:, 0, sl],
                                             rhs=ef_sb[:, 0, nsl],
                                             start=True, stop=False)
                            nc.tensor.matmul(acc[:, nsl], lhsT=z_sb[:, 1, sl],
                                             rhs=ef_sb[:, 1, nsl],
                                             start=False, stop=False)
                            nc.tensor.matmul(acc[:, nsl], lhsT=zh_sb[:, 0, sl],
                                             rhs=eb_sb[:, 0, nsl],
                                             start=False, stop=False)
                            nc.tensor.matmul(acc[:, nsl], lhsT=zh_sb[:, 1, sl],
                                             rhs=eb_sb[:, 1, nsl],
                                             start=False, stop=False)
                            if THREE_PASS:
                                nc.tensor.matmul(acc[:, nsl],
                                                 lhsT=zl_sb[:, 0, sl],
                                                 rhs=eh_sb[:, 0, nsl],
                                                 start=False, stop=False)
                                nc.tensor.matmul(acc[:, nsl],
                                                 lhsT=zl_sb[:, 1, sl],
                                                 rhs=eh_sb[:, 1, nsl],
                                                 start=False, stop=False)
                            nc.tensor.matmul(acc[:, nsl], lhsT=ones_row[:],
                                             rhs=bias_row[:, nsl],
                                             start=False, stop=True)
                        rm = rmpool.tile([TILE, K], f32, tag="rmt")
                        nc.vector.tensor_tensor_scan(
                            out=rm[:], data0=acc[:],
                            data1=dummy1.to_broadcast([TILE, K]),
                            initial=-3.0e38,
                            op0=mybir.AluOpType.max,
                            op1=mybir.AluOpType.bypass)
                        scr = scrpool.tile([TILE, K], bf16, tag="scrt")
                        kf = ipool.tile([TILE, 1], f32, tag="kf")
                        nc.scalar.activation(scr[:], rm[:], AF.Sign,
                                             bias=rm[:, K-1:K], scale=-1.0,
                                             accum_out=kf[:])
                        ku = ipool.tile([TILE, 1], u32, tag="ku")
                        nc.scalar.activation(ku[:], kf[:], AF.Copy)
                        g_sb = gpool.tile([TILE, D], f32, tag="gsb")
                        nc.gpsimd.indirect_dma_start(
                            out=g_sb[:], out_offset=None,
                            in_=etab[:],
                            in_offset=bass.IndirectOffsetOnAxis(
                                ap=ku[:, 0:1], axis=0))
                        nc.scalar.dma_start(out[tok0:tok0+TILE, :], g_sb[:])

            if reps > 1:
                with tc.For_i(0, reps, 1):
                    main_loop()
            else:
                main_loop()

    nc.compile()
    return nc


def _get_compiled():
    global _compiled
    if _compiled is None:
        _compiled = _build()
    return _compiled


def _make_in_maps(z: np.ndarray, E: np.ndarray):
    zflat = np.ascontiguousarray(z.reshape(TOK, D).astype(np.float32, copy=False))
    zT = np.ascontiguousarray(zflat.T)                      # [D, TOK] f32
    zhT = zT.astype(ml_dtypes.bfloat16)                     # [D, TOK] bf16

    Ef = np.ascontiguousarray(E.astype(np.float32, copy=False))  # [K, D]
    Ea = _r11(Ef)
    EbT = np.ascontiguousarray((Ef - Ea).T).astype(ml_dtypes.bfloat16)
    EfT = np.ascontiguousarray(Ef.T)                        # [D, K] f32

    brow = (-0.5 * (Ef.astype(np.float64) ** 2).sum(axis=1)).astype(
        np.float32)[None, :]                                # [1, K]

    extras = {}
    if THREE_PASS:
        zaT = _r11(zT)
        extras["ehT"] = np.ascontiguousarray(Ef.T.astype(ml_dtypes.bfloat16))
        extras["zlT"] = (zT - zaT).astype(ml_dtypes.bfloat16)

    in_maps = []
    for i in range(N_CORES):
        sl = slice(i * TOK_PC, (i + 1) * TOK_PC)
        m = {
            "zf": np.ascontiguousarray(zT[:, sl]),
            "zh": np.ascontiguousarray(zhT[:, sl]),
            "ef": EfT, "eb": EbT, "br": brow, "etab": Ef,
        }
        if THREE_PASS:
            m["zl"] = np.ascontiguousarray(extras["zlT"][:, sl])
            m["eh"] = extras["ehT"]
        in_maps.append(m)
    return in_maps


def kernel(z: np.ndarray, E: np.ndarray) -> np.ndarray:
    from concourse.bass_utils import run_bass_kernel_spmd

    nc = _get_compiled()
    in_maps = _make_in_maps(z, E)
    res = run_bass_kernel_spmd(nc, in_maps, core_ids=list(range(N_CORES)))
    outs = [res.results[i]["out"] for i in range(N_CORES)]
    return np.concatenate(outs, axis=0).reshape(B, U, D).astype(np.float32)


# revision 3
# speedup vs baseline: 1.5898x; 1.5898x over previous
"""VQ codebook nearest-neighbor kernel for Trainium2 (8 NeuronCores, data-parallel).

Problem: z [2048,64,256] f32, E [1024,256] f32 ->
         out[b,u,:] = E[argmin_k ||z[b,u]-E[k]||^2]

Strategy (v3):
  - Shard z along batch across 8 cores (16384 tokens each); replicate E.
  - argmin_k ||z-e_k||^2 == argmax_k (z.e_k - ||e_k||^2/2).  z.e_k via the
    3-term bf16 split (hi*hi + hi*lo + lo*hi), fp32 PSUM accumulate.
  - The -|e_k|^2/2 bias is PREFILLED into PSUM by the Scalar engine
    (activation Copy of a broadcast bias tile) and the 12 bf16 matmuls run
    with start=False, accumulating on top (PSUM has_written bits stay set
    from a one-time warmup matmul per PSUM buffer).  This removes the two
    K=1 bias matmuls from the PE's critical path.
  - DVE: InstMax (top-8) then InstMaxIndex straight out of PSUM; idx8[:,0]
    is the u32 argmax - no separate broadcast/convert instructions.
  - gpsimd indirect DMA gathers E rows; plain DMA stores the output.
"""
import numpy as np
import ml_dtypes

B, U, K, D = 2048, 64, 1024, 256
N_CORES = 8
TOK = B * U                    # 131072 tokens total
TOK_PC = TOK // N_CORES        # 16384 tokens per core
SUPER = 512                    # tokens per DMA super-tile
TILE = 128                     # tokens per compute tile
N_SUPER = TOK_PC // SUPER      # 32
TILES_PER_SUPER = SUPER // TILE  # 4
PSUM_BUFS = 3

BIAS_PREFILL = True            # False: fold bias via K=1 matmuls (baseline)

_compiled = None


def _build(reps: int = 1):
    from concourse import bacc
    import concourse.mybir as mybir
    import concourse.tile as tile
    import concourse.bass as bass
    import contextlib

    f32 = mybir.dt.float32
    f32r = mybir.dt.float32r
    bf16 = mybir.dt.bfloat16
    u32 = mybir.dt.uint32
    AF = mybir.ActivationFunctionType

    nc = bacc.Bacc("TRN2", target_bir_lowering=False, debug=False,
                   num_devices=N_CORES)

    zh = nc.declare_dram_parameter("zh", [D, TOK_PC], bf16, isOutput=False)
    zl = nc.declare_dram_parameter("zl", [D, TOK_PC], bf16, isOutput=False)
    eh = nc.declare_dram_parameter("eh", [D, K], bf16, isOutput=False)
    el = nc.declare_dram_parameter("el", [D, K], bf16, isOutput=False)
    br = nc.declare_dram_parameter("br", [1, K], f32, isOutput=False)
    etab = nc.declare_dram_parameter("etab", [K, D], f32, isOutput=False)
    out = nc.declare_dram_parameter("out", [TOK_PC, D], f32, isOutput=True)

    with tile.TileContext(nc) as tc:
        with contextlib.ExitStack() as ctx:
            const = ctx.enter_context(tc.tile_pool(name="const", bufs=1))
            zpool = ctx.enter_context(tc.tile_pool(name="zp", bufs=3))
            gpool = ctx.enter_context(tc.tile_pool(name="gp", bufs=4))
            ipool = ctx.enter_context(tc.tile_pool(name="ip", bufs=4))
            psum = ctx.enter_context(tc.tile_pool(name="ps", bufs=PSUM_BUFS,
                                                  space="PSUM"))
            pbias = ctx.enter_context(tc.tile_pool(name="pb", bufs=1,
                                                   space="PSUM"))

            # ---------------- one-time setup ----------------
            eh_sb = const.tile([128, 2, K], bf16, tag="ehsb")
            el_sb = const.tile([128, 2, K], bf16, tag="elsb")
            for c in range(2):
                nc.sync.dma_start(eh_sb[:, c, :], eh[c*128:(c+1)*128, :])
                nc.sync.dma_start(el_sb[:, c, :], el[c*128:(c+1)*128, :])
            br_sb = const.tile([1, K], f32, tag="brsb")
            nc.sync.dma_start(br_sb[:], br[:, :])
            br_r = const.tile([1, K], f32r, tag="brr")
            nc.vector.tensor_copy(br_r[:], br_sb[:])
            ones_f = const.tile([1, 128], f32, tag="onesf")
            nc.vector.memset(ones_f[:], 1.0)
            ones_row = const.tile([1, 128], f32r, tag="onesrow")
            nc.vector.tensor_copy(ones_row[:], ones_f[:])

            if BIAS_PREFILL:
                # bias_bcast [128, K] f32 in SBUF via ones-column matmul
                bias_ps = pbias.tile([TILE, K], f32, tag="biasps")
                for n in range(2):
                    nc.tensor.matmul(bias_ps[:, n*512:(n+1)*512],
                                     lhsT=ones_row[:],
                                     rhs=br_r[:, n*512:(n+1)*512],
                                     start=True, stop=True)
                bias_bc = const.tile([TILE, K], f32, tag="biasbc")
                nc.scalar.activation(bias_bc[:], bias_ps[:], AF.Copy)
                # warm up has_written bits on every acc PSUM buffer
                for _ in range(PSUM_BUFS):
                    acc0 = psum.tile([TILE, K], f32, tag="acc")
                    for n in range(2):
                        nc.tensor.matmul(acc0[:, n*512:(n+1)*512],
                                         lhsT=ones_row[:],
                                         rhs=br_r[:, n*512:(n+1)*512],
                                         start=True, stop=True)

            def main_loop():
                for s in range(N_SUPER):
                    zh_sb = zpool.tile([128, 2, SUPER], bf16, tag="zhsb")
                    zl_sb = zpool.tile([128, 2, SUPER], bf16, tag="zlsb")
                    for c in range(2):
                        nc.sync.dma_start(zh_sb[:, c, :],
                                          zh[c*128:(c+1)*128, s*SUPER:(s+1)*SUPER])
                        nc.sync.dma_start(zl_sb[:, c, :],
                                          zl[c*128:(c+1)*128, s*SUPER:(s+1)*SUPER])
                    for j in range(TILES_PER_SUPER):
                        tok0 = s * SUPER + j * TILE
                        sl = slice(j*TILE, (j+1)*TILE)
                        acc = psum.tile([TILE, K], f32, tag="acc")
                        if BIAS_PREFILL:
                            nc.scalar.activation(acc[:], bias_bc[:], AF.Copy)
                        for n in range(2):
                            nsl = slice(n*512, (n+1)*512)
                            first = not BIAS_PREFILL
                            mm = [(zh_sb, eh_sb), (zh_sb, el_sb), (zl_sb, eh_sb)]
                            cnt = 0
                            for (zz, ee) in mm:
                                for c in range(2):
                                    last = (cnt == 5) and BIAS_PREFILL
                                    nc.tensor.matmul(
                                        acc[:, nsl],
                                        lhsT=zz[:, c, sl],
                                        rhs=ee[:, c, nsl],
                                        start=(first and cnt == 0),
                                        stop=last,
                                        skip_group_check=BIAS_PREFILL)
                                    cnt += 1
                            if not BIAS_PREFILL:
                                nc.tensor.matmul(
                                    acc[:, nsl], lhsT=ones_row[:],
                                    rhs=br_r[:, nsl],
                                    start=False, stop=True)
                        vm8 = ipool.tile([TILE, 8], f32, tag="vm8")
                        nc.vector.max(vm8[:], acc[:])
                        idx8 = ipool.tile([TILE, 8], u32, tag="idx8")
                        nc.vector.max_index(out=idx8[:], in_max=vm8[:],
                                            in_values=acc[:])
                        g_sb = gpool.tile([TILE, D], f32, tag="gsb")
                        nc.gpsimd.indirect_dma_start(
                            out=g_sb[:], out_offset=None,
                            in_=etab[:],
                            in_offset=bass.IndirectOffsetOnAxis(
                                ap=idx8[:, 0:1], axis=0))
                        nc.sync.dma_start(out[tok0:tok0+TILE, :], g_sb[:])

            if reps > 1:
                with tc.For_i(0, reps, 1):
                    main_loop()
            else:
                main_loop()

    nc.compile()
    return nc


def _get_compiled():
    global _compiled
    if _compiled is None:
        _compiled = _build()
    return _compiled


def _make_in_maps(z: np.ndarray, E: np.ndarray):
    zf = np.ascontiguousarray(z.reshape(TOK, D).astype(np.float32, copy=False))
    zh32 = zf.astype(ml_dtypes.bfloat16)
    zl32 = (zf - zh32.astype(np.float32)).astype(ml_dtypes.bfloat16)
    Ef = np.ascontiguousarray(E.astype(np.float32, copy=False))
    Eh = Ef.astype(ml_dtypes.bfloat16)
    El = (Ef - Eh.astype(np.float32)).astype(ml_dtypes.bfloat16)

    ehT = np.ascontiguousarray(Eh.T)               # [D, K] bf16
    elT = np.ascontiguousarray(El.T)
    zhT = np.ascontiguousarray(zh32.T)             # [D, TOK] bf16
    zlT = np.ascontiguousarray(zl32.T)
    brow = (-0.5 * (Ef.astype(np.float64) ** 2).sum(axis=1)).astype(
        np.float32)[None, :]                       # [1, K]

    in_maps = []
    for i in range(N_CORES):
        sl = slice(i * TOK_PC, (i + 1) * TOK_PC)
        in_maps.append({
            "zh": np.ascontiguousarray(zhT[:, sl]),
            "zl": np.ascontiguousarray(zlT[:, sl]),
            "eh": ehT, "el": elT, "br": brow, "etab": Ef,
        })
    return in_maps


def kernel(z: np.ndarray, E: np.ndarray) -> np.ndarray:
    from concourse.bass_utils import run_bass_kernel_spmd

    nc = _get_compiled()
    in_maps = _make_in_maps(z, E)
    res = run_bass_kernel_spmd(nc, in_maps, core_ids=list(range(N_CORES)))
    outs = [res.results[i]["out"] for i in range(N_CORES)]
    return np.concatenate(outs, axis=0).reshape(B, U, D).astype(np.float32)
